# revision 47
# baseline (speedup 1.0000x reference)
"""ColBERT intra-batch MaxSim scoring kernel for 8 Trainium2 NeuronCores.

Math (see reference):
  Q = l2norm(q_hidden @ W.T)                       [B, LQ, DIM]
  D = l2norm(d_hidden @ W.T); D masked             [B, LD, DIM]
  sim[b,c,q,k] = Q[b,q]·D[c,k]; masked k -> -inf
  out[b,c] = sum_q max_k sim

Sharding: docs (dim c) are sharded 16-per-core; q_hidden/W replicated.
Each core computes its [B, 16] slice of the score matrix.

Device-side structure:
  * Host pre-transposes activations to [HID, tokens] and converts to bf16
    (halves HBM traffic; verified 9.6e-4 rel err vs 2e-2 budget).
  * The doc mask is folded away on the host: each doc's valid tokens are
    gathered to the front and the tail is padded with copies of the doc's
    first valid token, so the device kernel needs no masking.
  * All input DMAs are issued on the sync queue in priority order (wt,
    dT chunks, then qT column groups).  One HWDGE ring = strict FIFO, so
    dT gets full bandwidth first and the doc pipeline starts ~15us in;
    qT groups trickle in behind at the rate the sim tiles consume them.
  * Q is NOT normalized before the sim matmul: max_k is invariant under a
    positive per-query scale, so 1/|Q| is folded into the block-ones
    lhsT of the final query-sum matmul.
  * D norms, chunk-pipelined: Square (ACT) -> M=8 ones-matmul sumsq (PE,
    full rate) -> approx reciprocal (DVE, straight from PSUM) -> sqrt with
    free bf16 cast (ACT) -> K=8 ones broadcast matmul (PE) -> multiply
    (DVE).  dT ships as fp8-e4m3 (d_hidden is N(0,1) — 6.7e-3 rel err
    total, vs the 2e-2 budget), halving the head-critical DMA.
  * Sim phase: with two PSUM slots a tile is bounded by its two halves'
    consumers when both land on one engine, so most tiles are "hybrid":
    half h0 -> DVE direct grouped reduce_max; half h1 -> ACT copy to SBUF
    bf16, folded 2 tiles later on the DVE via two 2x-rate TT-max levels +
    a short reduce (delayed so the DVE never waits on ACT).  Interleaved
    full-fold tiles shift load toward ACT to balance the engines.
  * ACT spline tables for Square/Sqrt are warmed at t=0 so the first
    normalize step doesn't eat the ~2.6us table-load latency.
"""

import os

import numpy as np

B, LQ, LD, HID, DIM = 128, 32, 256, 768, 128
NCORES = 8
DPC = B // NCORES          # docs per core
TQ = B * LQ                # total query tokens
KC = HID // 128            # contraction chunks for the projection

SIM_MODE = os.environ.get("KERNEL_SIM_MODE", "bf16")
# With only two PSUM slots, a tile's wall time is bounded by its two
# halves' consumers when they land on ONE engine (2 DVE reduces for a
# direct tile, 2 ACT converts for a fold tile).  Hybrid tiles send h0 to
# the DVE (direct reduce) and h1 to ACT (convert + delayed DVE TT-max
# fold), so the two consumers overlap.  A few full-fold tiles rebalance
# total load toward ACT.  D = direct, H = hybrid, F = full-fold.
FOLD = os.environ.get("KERNEL_FOLD", "1") == "1"


def _tile_kind(t):
    if not FOLD:
        return "D"
    if t < 2:
        return "D"
    # all-hybrid: ACT runs far under saturation (~44us vs DVE ~78us), so
    # converts are always ready and the DVE never stalls on cross-engine
    # coupling; the window is purely DVE-paced at ~2.4us/tile.  Mixing in
    # full-fold tiles was tried at 15/15 and 7/23 — both regressed (their
    # two serial ACT converts couple the PSUM slot pipeline).
    return "H"

# qT column groups, in DMA priority order (first groups smaller so the
# first sim tiles can start as early as possible)
QGROUPS = [(0, 512), (512, 512), (1024, 1024), (2048, 1024), (3072, 1024)]


def _chunks(total, step):
    """[(off, len)] cut at `step` boundaries — a matmul's PSUM output must
    stay inside a single 512-float bank, so chunks may never straddle one."""
    return [(o, min(step, total - o)) for o in range(0, total, step)]


def _qgroup_of(j):
    """(group index, column offset within group) for 512-col chunk j."""
    off = j * 512
    for gi, (go, gw) in enumerate(QGROUPS):
        if go <= off < go + gw:
            return gi, off - go
    raise ValueError(j)


def _build_program(NV):
    import concourse.bass as bass  # noqa: F401
    import concourse.tile as tile
    from concourse import bacc, mybir

    f32 = mybir.dt.float32
    bf16 = mybir.dt.bfloat16
    AF = mybir.ActivationFunctionType
    AX = mybir.AxisListType
    ALU = mybir.AluOpType

    proj_dt = bf16
    sim_dt = {"bf16": bf16, "f32": f32}[SIM_MODE]
    sq_dt = bf16

    NVT = DPC * NV          # compacted doc tokens per core
    NVH = NVT // 2          # half (8 docs) — one PSUM sim tile
    NQCH = TQ // 512        # q-projection column chunks
    NTT = TQ // 128         # sim lhsT tiles (query-token tiles)
    BPT = 128 // LQ         # batch entries per query-token tile
    d_chunks = _chunks(NVT, 512)   # d-projection column chunks
    s_chunks = _chunks(NVH, 512)   # sim matmul N chunks per half

    nc = bacc.Bacc(
        "TRN2",
        target_bir_lowering=False,
        debug=False,
        num_devices=NCORES,
    )

    d8 = mybir.dt.float8e4
    qT_d = nc.dram_tensor("qT", [HID, TQ], proj_dt, kind="ExternalInput")
    dT_d = nc.dram_tensor("dT", [HID, NVT], d8, kind="ExternalInput")
    wT_d = nc.dram_tensor("wT", [128, KC, DIM], proj_dt, kind="ExternalInput")
    qso_d = nc.dram_tensor("qso", [128, BPT], f32, kind="ExternalInput")
    onescol_d = nc.dram_tensor("onescol", [128, 1], sq_dt, kind="ExternalInput")
    out_d = nc.dram_tensor("out", [B, DPC], f32, kind="ExternalOutput")

    # [HID, t] rows seen as (k, p): row = k*128 + p
    qT_v = qT_d[:, :].rearrange("(k p) t -> p k t", p=128)

    with tile.TileContext(nc) as tc, tc.tile_pool(name="persist", bufs=1) as per:
        # --- constants + persistent SBUF tensors ---------------------------
        wt = per.tile([128, KC, DIM], proj_dt, name="wt")
        qso = per.tile([128, BPT], f32, name="qso")
        onescol = per.tile([128, 1], sq_dt, name="onescol")
        onescol8 = per.tile([128, 8], sq_dt, name="onescol8")
        oneeighth = per.tile([8, 128], sq_dt, name="oneeighth")
        warm = per.tile([1, 16], f32, name="warm")
        QT = per.tile([128, TQ], sim_dt, name="QT")       # q-proj [d, t] unnormalized
        DTnA = per.tile([128, NVH], sim_dt, name="DTnA")  # normalized d-proj h0
        DTnB = per.tile([128, NVT - NVH], sim_dt, name="DTnB")  # h1
        invnQ = per.tile([128, NTT], f32, name="invnQ")   # 1/|Q| per query token
        normQ = per.tile([128, NTT], f32, name="normQ")
        lhsQ = per.tile([128, NTT, BPT], f32, name="lhsQ")  # blockones * 1/|Q|
        rowtmp8 = per.tile([8, NVT], f32, name="rowtmp8")    # 1/ssq, 8 rows
        invnD8 = per.tile([8, NVT], sq_dt, name="invnD8")    # 1/|D| bf16, 8 rows
        outstage = per.tile([BPT, NTT * DPC], f32, name="outstage")
        sqqA = per.tile([128, 512], sq_dt, name="sqqA")
        sqqB = per.tile([128, 512], sq_dt, name="sqqB")
        dts = [per.tile([128, NVT], d8, name=f"dt{k}") for k in range(KC)]
        qtg = [
            per.tile([128, KC, gw], proj_dt, name=f"qtg{gi}")
            for gi, (_, gw) in enumerate(QGROUPS)
        ]

        # input DMAs: one ring (sync), strict priority order
        nc.sync.dma_start(wt[:], wT_d[:, :, :])
        for k in range(KC):
            nc.sync.dma_start(dts[k][:], dT_d[k * 128:(k + 1) * 128, :])
        for gi, (go, gw) in enumerate(QGROUPS):
            nc.sync.dma_start(qtg[gi][:, :, :], qT_v[:, :, go:go + gw])
        # tiny constants off the critical ring (SWDGE)
        nc.gpsimd.dma_start(qso[:], qso_d[:, :])
        nc.gpsimd.dma_start(onescol[:], onescol_d[:, :])

        # warm the ACT spline tables while DMAs are in flight
        nc.vector.memset(warm[:], 1.0)
        nc.vector.memset(onescol8[:], 1.0)
        nc.vector.memset(oneeighth[:], 0.125)
        nc.scalar.activation(warm[:], warm[:], AF.Square)
        nc.scalar.activation(warm[:], warm[:], AF.Sqrt)

        # ---------------- phase D: project doc tokens ----------------------
        # k-outer accumulation into one wide PSUM tensor so compute starts
        # as soon as the first dT k-chunk lands.
        with (
            tc.tile_pool(name="psD", bufs=1, space="PSUM") as psD,
            tc.tile_pool(name="ssD", bufs=1, space="PSUM") as ssD,
            tc.tile_pool(name="sqD_pool", bufs=2) as sqD_pool,
            tc.tile_pool(name="bc_pool", bufs=2) as bc_pool,
            tc.tile_pool(name="psB", bufs=1, space="PSUM") as psB,
            tc.tile_pool(name="psqP", bufs=1, space="PSUM") as psqP,
        ):
            # psd split per 512-chunk so each chunk's PSUM bank frees right
            # after its DTn multiply (the Q-projection PSUM reuses them)
            psds = [
                psD.tile([128, ln], f32, name=f"psd{ci}")
                for ci, (off, ln) in enumerate(d_chunks)
            ]
            # first group = one chunk: its k=5 accumulation completes right
            # as dT5 lands, so the ACT-serial norm chain starts ~4.5us sooner
            cgroups = [[0], [1, 2], [3, 4]]
            for cg in cgroups:
                for k in range(KC):
                    for ci in cg:
                        off, ln = d_chunks[ci]
                        nc.tensor.matmul(
                            psds[ci][:, :ln],
                            wt[:, k, :],
                            dts[k][:, off:off + ln],
                            start=(k == 0),
                            stop=(k == KC - 1),
                        )
            # chunk-granular norm chain: Square (ACT) -> M=8 ones matmul (PE,
            # full-rate) -> sqrt row straight from PSUM (ACT) -> ~51-ULP
            # reciprocal (DVE); stages pipeline across the 5 chunks
            for ci, (off, ln) in enumerate(d_chunks):
                sl = slice(off, off + ln)
                sq = sqD_pool.tile([128, 512], sq_dt, name="sqd", tag="sq")
                nc.scalar.activation(sq[:, :ln], psds[ci][:, :ln], AF.Square)
                ssd = ssD.tile([8, 512], f32, name="ssd", tag="ssd")
                nc.tensor.matmul(
                    ssd[:, :ln], onescol8[:], sq[:, :ln], start=True, stop=True
                )
                nc.vector.reciprocal_approx_fast(
                    rowtmp8[:, sl], ssd[:, :ln]
                )
                nc.scalar.activation(invnD8[:, sl], rowtmp8[:, sl], AF.Sqrt)

            # Q-projection chunk 0 into its own PSUM bank (runs in the PE's
            # DMA-wait gaps); its QT copy + square land on ACT just before
            # the bc copies so sim tile 0 can fire the moment DTn is done
            psq01 = {}

            def qproj_early(j):
                psq = psqP.tile([128, 512], f32, name=f"psq{j}", tag="psq")
                psq01[j] = psq
                gi, r = _qgroup_of(j)
                for k in range(KC):
                    nc.tensor.matmul(
                        psq[:], wt[:, k, :], qtg[gi][:, k, r:r + 512],
                        start=(k == 0), stop=(k == KC - 1),
                    )

            def qcopy_early(j, sqq):
                sl = slice(j * 512, (j + 1) * 512)
                nc.scalar.copy(QT[:, sl], psq01[j][:])
                nc.scalar.activation(sqq[:], psq01[j][:], AF.Square)

            qproj_early(0)
            qcopy_early(0, sqqA)

            # broadcast 1/|D| across partitions and scale D straight out of
            # the projection PSUM (each psd chunk dies at its multiply)
            for ci, (off, ln) in enumerate(d_chunks):
                sl = slice(off, off + ln)
                psb = psB.tile([128, 512], f32, name="psb", tag="psb")
                nc.tensor.matmul(
                    psb[:, :ln], oneeighth[:], invnD8[:, sl], start=True, stop=True
                )
                bc = bc_pool.tile([128, 512], f32, name="bcast_sb", tag="bc")
                if ci < 3:
                    nc.scalar.copy(bc[:, :ln], psb[:, :ln])
                else:
                    nc.vector.tensor_scalar_mul(bc[:, :ln], psb[:, :ln], 1.0)
                segs = [(s, e) for s, e in
                        [(off, min(off + ln, NVH)), (max(off, NVH), off + ln)]
                        if e > s]
                for (s, e) in segs:
                    dst = (DTnA[:, s:e] if e <= NVH
                           else DTnB[:, s - NVH:e - NVH])
                    nc.vector.tensor_tensor(
                        dst, psds[ci][:, s - off:e - off],
                        bc[:, s - off:e - off], op=ALU.mult,
                    )
            qproj_early(1)
            qcopy_early(1, sqqB)

        # ---------- phase Q+S: project query chunks, sim tiles interleaved --
        # Q-projection chunk j feeds sim tiles t=4j..4j+3; chunks are traced
        # two groups ahead of their sim tiles so the PE never starves the DVE
        # reduce pipeline.  pssim is a single 5-bank tensor whose two halves
        # ping-pong between PE writes and DVE reduces.
        with (
            tc.tile_pool(name="psQS", bufs=2, space="PSUM") as psQS,
            tc.tile_pool(name="ssQ", bufs=1, space="PSUM") as ssQ,
            tc.tile_pool(name="sqQ_pool", bufs=2) as sqQ_pool,
            tc.tile_pool(name="psO", bufs=1, space="PSUM") as psO,
            tc.tile_pool(name="m_pool", bufs=8) as m_pool,
            tc.tile_pool(name="fold_pool", bufs=5) as fold_pool,
        ):
            ssq = ssQ.tile([128, NTT], f32, name="ssq")
            psout = psO.tile([BPT, NTT * DPC], f32, name="psout")

            psq_live = {}

            def project_mm(j, ks):
                gi, r = _qgroup_of(j)
                if j not in psq_live:
                    psq_live[j] = psQS.tile([128, NVH], f32, name="psq", tag="big")
                psq = psq_live[j]
                for k in ks:
                    nc.tensor.matmul(
                        psq[:, 0:512],
                        wt[:, k, :],
                        qtg[gi][:, k, r:r + 512],
                        start=(k == 0),
                        stop=(k == KC - 1),
                    )

            def qnorm(j, sq):
                # ssq matmuls + per-chunk 1/|Q| and the weighted lhsT
                for s in range(4):
                    col = j * 4 + s
                    nc.tensor.matmul(
                        ssq[:, col:col + 1],
                        sq[:, s * 128:(s + 1) * 128],
                        onescol[:],
                        start=True,
                        stop=True,
                    )
                csl = slice(j * 4, (j + 1) * 4)
                nc.scalar.activation(normQ[:, csl], ssq[:, csl], AF.Sqrt)
                nc.vector.reciprocal(invnQ[:, csl], normQ[:, csl])
                nc.vector.tensor_tensor(
                    lhsQ[:, csl, :],
                    qso[:].unsqueeze(1).broadcast_to((128, 4, BPT)),
                    invnQ[:, csl].unsqueeze(2).broadcast_to((128, 4, BPT)),
                    op=ALU.mult,
                )

            def project(j):
                sl = slice(j * 512, (j + 1) * 512)
                psq = psq_live.pop(j)
                nc.scalar.copy(QT[:, sl], psq[:, 0:512])
                sq = sqQ_pool.tile([128, 512], sq_dt, name="sqq", tag="sqq")
                nc.scalar.activation(sq[:], psq[:, 0:512], AF.Square)
                qnorm(j, sq)

            def psout_mm(t, mall):
                nc.tensor.matmul(
                    psout[:, t * DPC:(t + 1) * DPC],
                    lhsQ[:, t, :],
                    mall[:],
                    start=True,
                    stop=True,
                )

            def fold_levels(sv_tiles, g, out, tag):
                # two 2x-rate TT-max fold levels then a short 1x reduce over
                # [128, g, NV/4]; sv_tiles is a (g*NV)-wide bf16 SBUF region
                v2, v4 = NV // 2, NV // 4
                l1 = fold_pool.tile([128, g * v2], sim_dt, name=f"l1{tag}",
                                    tag=f"l1{tag}")
                sv = sv_tiles.rearrange("p (g v) -> p g v", v=NV)
                nc.vector.tensor_tensor(
                    l1[:].rearrange("p (g v) -> p g v", v=v2),
                    sv[:, :, 0:v2], sv[:, :, v2:NV], op=ALU.max,
                )
                l2 = fold_pool.tile([128, g * v4], sim_dt, name=f"l2{tag}",
                                    tag=f"l2{tag}")
                lv = l1[:].rearrange("p (g v) -> p g v", v=v2)
                nc.vector.tensor_tensor(
                    l2[:].rearrange("p (g v) -> p g v", v=v4),
                    lv[:, :, 0:v4], lv[:, :, v4:v2], op=ALU.max,
                )
                nc.vector.reduce_max(
                    out, l2[:].rearrange("p (g v) -> p g v", v=v4), axis=AX.X
                )

            def fold_finish(t, kind, sbt, mall):
                # issued a couple of tiles late so the DVE's inputs are
                # always ready (no cross-engine just-in-time stalls)
                if kind == "F":
                    fold_levels(sbt[:], DPC, mall[:], "f")
                else:  # H: h1 only (docs 8..15)
                    fold_levels(sbt[:], DPC // 2, mall[:, DPC // 2:DPC], "h")
                psout_mm(t, mall)

            def simtile(t, mid=None):
                lq = QT[:, t * 128:(t + 1) * 128]
                kind = _tile_kind(t)
                mall = m_pool.tile([128, DPC], f32, name="mall", tag="mall")
                if kind == "F":
                    sbt = fold_pool.tile([128, NVT], sim_dt, name="sbt", tag="sbt")
                elif kind == "H":
                    sbt = fold_pool.tile([128, NVH], sim_dt, name="sbh", tag="sbh")
                for h in range(2):
                    dtn = DTnA if h == 0 else DTnB
                    ps = psQS.tile([128, NVH], f32, name="pssim", tag="big")
                    for (off, ln) in s_chunks:
                        nc.tensor.matmul(
                            ps[:, off:off + ln],
                            lq,
                            dtn[:, off:off + ln],
                            start=True,
                            stop=True,
                        )
                    if kind == "F" or (kind == "H" and h == 1):
                        # evict to SBUF bf16 on ACT; DVE folds later
                        off0 = base if kind == "F" else 0
                        nc.scalar.copy(sbt[:, off0:off0 + NVH], ps[:])
                    else:
                        nc.vector.reduce_max(
                            mall[:, h * (DPC // 2):(h + 1) * (DPC // 2)],
                            ps[:].rearrange("p (g v) -> p g v", v=NV),
                            axis=AX.X,
                        )
                    if mid:
                        mid(h)
                if kind == "D":
                    psout_mm(t, mall)
                    return None
                return (t, kind, sbt, mall)

            # chunks 0/1 were projected + copied during phase D; finish norms
            qnorm(0, sqqA)
            pending = []

            def run_tile(t, mid=None):
                p = simtile(t, mid)
                if p is not None:
                    pending.append(p)
                depth = 3 if t < 29 else 1
                while len(pending) > depth:
                    fold_finish(*pending.pop(0))

            run_tile(0)
            qnorm(1, sqqB)
            run_tile(1)
            for t in range(2, NTT):
                j, ti = divmod(t, 4)
                mid = None
                if ti == 2 and j + 2 < NQCH:
                    # half the next-next chunk's projection between each sim
                    # half so the PE pause never starves the DVE
                    def mid(h, _j=j + 2):
                        project_mm(_j, range(3 * h, 3 * h + 3))

                run_tile(t, mid)
                if ti == 2 and j + 2 < NQCH:
                    project(j + 2)
            for p in pending:
                fold_finish(*p)
            nc.vector.tensor_scalar_mul(outstage[:], psout[:], 1.0)
            nc.sync.dma_start(
                out_d[:, :].rearrange("(t f) c -> f t c", f=BPT),
                outstage[:].rearrange("f (t c) -> f t c", c=DPC),
            )

    nc.compile()
    return nc


def _host_prep(q_hidden, d_hidden, W, d_mask):
    import ml_dtypes

    bf = ml_dtypes.bfloat16
    q = np.ascontiguousarray(np.asarray(q_hidden, dtype=np.float32))
    d = np.ascontiguousarray(np.asarray(d_hidden, dtype=np.float32))
    w = np.ascontiguousarray(np.asarray(W, dtype=np.float32))
    mask = np.asarray(d_mask, dtype=bool)

    nv = mask.sum(axis=1)
    NV = int(-(-max(int(nv.max()), 16) // 8) * 8)
    NV = min(NV, ((LD + 7) // 8) * 8)

    # per-doc gather indices: valid tokens first, padded with the first
    # valid token (duplicates never change a max)
    idx = np.zeros((B, NV), dtype=np.intp)
    for c in range(B):
        v = np.flatnonzero(mask[c])
        row = np.full(NV, v[0], dtype=np.intp)
        row[:min(len(v), NV)] = v[:NV]
        idx[c] = row

    dG = d[np.arange(B)[:, None], idx, :]          # [B, NV, HID]

    qT = np.ascontiguousarray(q.reshape(TQ, HID).T.astype(bf))   # [HID, TQ]
    # W.T rearranged so the [128, KC, DIM] SBUF tile is one contiguous DMA:
    # wTp[p, k, d] = W[d, k*128+p]
    wT = np.ascontiguousarray(
        w.T.reshape(KC, 128, DIM).transpose(1, 0, 2).astype(bf)
    )
    f8 = ml_dtypes.float8_e4m3
    dT_cores = []
    for m in range(NCORES):
        blk = dG[m * DPC:(m + 1) * DPC].reshape(DPC * NV, HID)
        dT_cores.append(np.ascontiguousarray(blk.T.astype(f8)))  # [HID, DPC*NV]

    qso = np.zeros((128, 128 // LQ), dtype=np.float32)
    for p in range(128):
        qso[p, p // LQ] = 1.0
    onescol = np.ones((128, 1), dtype=bf)
    return NV, qT, wT, dT_cores, qso, onescol


def kernel(q_hidden, d_hidden, W, d_mask):
    from concourse.bass_utils import run_bass_kernel_spmd

    NV, qT, wT, dT_cores, qso, onescol = _host_prep(
        q_hidden, d_hidden, W, d_mask
    )
    nc = _build_program(NV)

    in_maps = [
        {
            "qT": qT,
            "dT": dT_cores[m],
            "wT": wT,
            "qso": qso,
            "onescol": onescol,
        }
        for m in range(NCORES)
    ]
    res = run_bass_kernel_spmd(nc, in_maps, core_ids=list(range(NCORES)))
    out = np.concatenate(
        [res.results[m]["out"] for m in range(NCORES)], axis=1
    )
    return np.ascontiguousarray(out.astype(np.float32))


# revision 48
# speedup vs baseline: 1.0052x; 1.0052x over previous
"""ColBERT intra-batch MaxSim scoring kernel for 8 Trainium2 NeuronCores.

Math (see reference):
  Q = l2norm(q_hidden @ W.T)                       [B, LQ, DIM]
  D = l2norm(d_hidden @ W.T); D masked             [B, LD, DIM]
  sim[b,c,q,k] = Q[b,q]·D[c,k]; masked k -> -inf
  out[b,c] = sum_q max_k sim

Sharding: docs (dim c) are sharded 16-per-core; q_hidden/W replicated.
Each core computes its [B, 16] slice of the score matrix.

Device-side structure:
  * Host pre-transposes activations to [HID, tokens] and converts to bf16
    (halves HBM traffic; verified 9.6e-4 rel err vs 2e-2 budget).
  * The doc mask is folded away on the host: each doc's valid tokens are
    gathered to the front and the tail is padded with copies of the doc's
    first valid token, so the device kernel needs no masking.
  * All input DMAs are issued on the sync queue in priority order (wt,
    dT chunks, then qT column groups).  One HWDGE ring = strict FIFO, so
    dT gets full bandwidth first and the doc pipeline starts ~15us in;
    qT groups trickle in behind at the rate the sim tiles consume them.
  * Q is NOT normalized before the sim matmul: max_k is invariant under a
    positive per-query scale, so 1/|Q| is folded into the block-ones
    lhsT of the final query-sum matmul.
  * D norms, chunk-pipelined: Square (ACT) -> M=8 ones-matmul sumsq (PE,
    full rate) -> approx reciprocal (DVE, straight from PSUM) -> sqrt with
    free bf16 cast (ACT) -> K=8 ones broadcast matmul (PE) -> multiply
    (DVE).  dT ships as fp8-e4m3 (d_hidden is N(0,1) — 6.7e-3 rel err
    total, vs the 2e-2 budget), halving the head-critical DMA.
  * Sim phase: with two PSUM slots a tile is bounded by its two halves'
    consumers when both land on one engine, so most tiles are "hybrid":
    half h0 -> DVE direct grouped reduce_max; half h1 -> ACT copy to SBUF
    bf16, folded 2 tiles later on the DVE via two 2x-rate TT-max levels +
    a short reduce (delayed so the DVE never waits on ACT).  Interleaved
    full-fold tiles shift load toward ACT to balance the engines.
  * ACT spline tables for Square/Sqrt are warmed at t=0 so the first
    normalize step doesn't eat the ~2.6us table-load latency.
"""

import os

import numpy as np

B, LQ, LD, HID, DIM = 128, 32, 256, 768, 128
NCORES = 8
DPC = B // NCORES          # docs per core
TQ = B * LQ                # total query tokens
KC = HID // 128            # contraction chunks for the projection

SIM_MODE = os.environ.get("KERNEL_SIM_MODE", "bf16")
# With only two PSUM slots, a tile's wall time is bounded by its two
# halves' consumers when they land on ONE engine (2 DVE reduces for a
# direct tile, 2 ACT converts for a fold tile).  Hybrid tiles send h0 to
# the DVE (direct reduce) and h1 to ACT (convert + delayed DVE TT-max
# fold), so the two consumers overlap.  A few full-fold tiles rebalance
# total load toward ACT.  D = direct, H = hybrid, F = full-fold.
FOLD = os.environ.get("KERNEL_FOLD", "1") == "1"


def _tile_kind(t):
    if not FOLD:
        return "D"
    if t < 2:
        return "D"
    # all-hybrid: ACT runs far under saturation (~44us vs DVE ~78us), so
    # converts are always ready and the DVE never stalls on cross-engine
    # coupling; the window is purely DVE-paced at ~2.4us/tile.  Mixing in
    # full-fold tiles was tried at 15/15 and 7/23 — both regressed (their
    # two serial ACT converts couple the PSUM slot pipeline).
    return "H"

# qT column groups, in DMA priority order (first groups smaller so the
# first sim tiles can start as early as possible)
QGROUPS = [(0, 512), (512, 512), (1024, 1024), (2048, 1024), (3072, 1024)]


def _chunks(total, step):
    """[(off, len)] cut at `step` boundaries — a matmul's PSUM output must
    stay inside a single 512-float bank, so chunks may never straddle one."""
    return [(o, min(step, total - o)) for o in range(0, total, step)]


def _qgroup_of(j):
    """(group index, column offset within group) for 512-col chunk j."""
    off = j * 512
    for gi, (go, gw) in enumerate(QGROUPS):
        if go <= off < go + gw:
            return gi, off - go
    raise ValueError(j)


def _build_program(NV):
    import concourse.bass as bass  # noqa: F401
    import concourse.tile as tile
    from concourse import bacc, mybir

    f32 = mybir.dt.float32
    bf16 = mybir.dt.bfloat16
    AF = mybir.ActivationFunctionType
    AX = mybir.AxisListType
    ALU = mybir.AluOpType

    proj_dt = bf16
    sim_dt = {"bf16": bf16, "f32": f32}[SIM_MODE]
    sq_dt = bf16

    NVT = DPC * NV          # compacted doc tokens per core
    NVH = NVT // 2          # half (8 docs) — one PSUM sim tile
    NQCH = TQ // 512        # q-projection column chunks
    NTT = TQ // 128         # sim lhsT tiles (query-token tiles)
    BPT = 128 // LQ         # batch entries per query-token tile
    d_chunks = _chunks(NVT, 512)   # d-projection column chunks
    s_chunks = _chunks(NVH, 512)   # sim matmul N chunks per half

    nc = bacc.Bacc(
        "TRN2",
        target_bir_lowering=False,
        debug=False,
        num_devices=NCORES,
    )

    d8 = mybir.dt.float8e4
    qT_d = nc.dram_tensor("qT", [HID, TQ], proj_dt, kind="ExternalInput")
    dT_d = nc.dram_tensor("dT", [HID, NVT], d8, kind="ExternalInput")
    wT_d = nc.dram_tensor("wT", [128, KC, DIM], proj_dt, kind="ExternalInput")
    qso_d = nc.dram_tensor("qso", [128, BPT], f32, kind="ExternalInput")
    onescol_d = nc.dram_tensor("onescol", [128, 1], sq_dt, kind="ExternalInput")
    out_d = nc.dram_tensor("out", [B, DPC], f32, kind="ExternalOutput")

    # [HID, t] rows seen as (k, p): row = k*128 + p
    qT_v = qT_d[:, :].rearrange("(k p) t -> p k t", p=128)

    with tile.TileContext(nc) as tc, tc.tile_pool(name="persist", bufs=1) as per:
        # --- constants + persistent SBUF tensors ---------------------------
        wt = per.tile([128, KC, DIM], proj_dt, name="wt")
        qso = per.tile([128, BPT], f32, name="qso")
        onescol = per.tile([128, 1], sq_dt, name="onescol")
        onescol8 = per.tile([128, 8], sq_dt, name="onescol8")
        oneeighth = per.tile([8, 128], sq_dt, name="oneeighth")
        warm = per.tile([1, 16], f32, name="warm")
        QT = per.tile([128, TQ], sim_dt, name="QT")       # q-proj [d, t] unnormalized
        DTn = per.tile([128, NVT], sim_dt, name="DTn")    # normalized d-proj
        invnQ = per.tile([128, NTT], f32, name="invnQ")   # 1/|Q| per query token
        normQ = per.tile([128, NTT], f32, name="normQ")
        lhsQ = per.tile([128, NTT, BPT], f32, name="lhsQ")  # blockones * 1/|Q|
        rowtmp8 = per.tile([8, NVT], f32, name="rowtmp8")    # 1/ssq, 8 rows
        invnD8 = per.tile([8, NVT], sq_dt, name="invnD8")    # 1/|D| bf16, 8 rows
        outstage = per.tile([BPT, NTT * DPC], f32, name="outstage")
        sqqA = per.tile([128, 512], sq_dt, name="sqqA")
        sqqB = per.tile([128, 512], sq_dt, name="sqqB")
        dts = [per.tile([128, NVT], d8, name=f"dt{k}") for k in range(KC)]
        qtg = [
            per.tile([128, KC, gw], proj_dt, name=f"qtg{gi}")
            for gi, (_, gw) in enumerate(QGROUPS)
        ]

        # input DMAs: one ring (sync), strict priority order
        nc.sync.dma_start(wt[:], wT_d[:, :, :])
        for k in range(KC):
            nc.sync.dma_start(dts[k][:], dT_d[k * 128:(k + 1) * 128, :])
        for gi, (go, gw) in enumerate(QGROUPS):
            nc.sync.dma_start(qtg[gi][:, :, :], qT_v[:, :, go:go + gw])
        # tiny constants off the critical ring (SWDGE)
        nc.gpsimd.dma_start(qso[:], qso_d[:, :])
        nc.gpsimd.dma_start(onescol[:], onescol_d[:, :])

        # warm the ACT spline tables while DMAs are in flight
        nc.vector.memset(warm[:], 1.0)
        nc.vector.memset(onescol8[:], 1.0)
        nc.vector.memset(oneeighth[:], 0.125)
        nc.scalar.activation(warm[:], warm[:], AF.Square)
        nc.scalar.activation(warm[:], warm[:], AF.Sqrt)

        # ---------------- phase D: project doc tokens ----------------------
        # k-outer accumulation into one wide PSUM tensor so compute starts
        # as soon as the first dT k-chunk lands.
        with (
            tc.tile_pool(name="psD", bufs=1, space="PSUM") as psD,
            tc.tile_pool(name="ssD", bufs=1, space="PSUM") as ssD,
            tc.tile_pool(name="sqD_pool", bufs=2) as sqD_pool,
            tc.tile_pool(name="bc_pool", bufs=2) as bc_pool,
            tc.tile_pool(name="psB", bufs=1, space="PSUM") as psB,
            tc.tile_pool(name="psqP", bufs=1, space="PSUM") as psqP,
        ):
            # psd split per 512-chunk so each chunk's PSUM bank frees right
            # after its DTn multiply (the Q-projection PSUM reuses them)
            psds = [
                psD.tile([128, ln], f32, name=f"psd{ci}")
                for ci, (off, ln) in enumerate(d_chunks)
            ]
            # first group = one chunk: its k=5 accumulation completes right
            # as dT5 lands, so the ACT-serial norm chain starts ~4.5us sooner
            cgroups = [[0], [1, 2], [3, 4]]
            for cg in cgroups:
                for k in range(KC):
                    for ci in cg:
                        off, ln = d_chunks[ci]
                        nc.tensor.matmul(
                            psds[ci][:, :ln],
                            wt[:, k, :],
                            dts[k][:, off:off + ln],
                            start=(k == 0),
                            stop=(k == KC - 1),
                        )
            # chunk-granular norm chain: Square (ACT) -> M=8 ones matmul (PE,
            # full-rate) -> sqrt row straight from PSUM (ACT) -> ~51-ULP
            # reciprocal (DVE); stages pipeline across the 5 chunks
            for ci, (off, ln) in enumerate(d_chunks):
                sl = slice(off, off + ln)
                sq = sqD_pool.tile([128, 512], sq_dt, name="sqd", tag="sq")
                nc.scalar.activation(sq[:, :ln], psds[ci][:, :ln], AF.Square)
                ssd = ssD.tile([8, 512], f32, name="ssd", tag="ssd")
                nc.tensor.matmul(
                    ssd[:, :ln], onescol8[:], sq[:, :ln], start=True, stop=True
                )
                nc.vector.reciprocal_approx_fast(
                    rowtmp8[:, sl], ssd[:, :ln]
                )
                nc.scalar.activation(invnD8[:, sl], rowtmp8[:, sl], AF.Sqrt)

            # Q-projection chunk 0 into its own PSUM bank (runs in the PE's
            # DMA-wait gaps); its QT copy + square land on ACT just before
            # the bc copies so sim tile 0 can fire the moment DTn is done
            psq01 = {}

            def qproj_early(j):
                psq = psqP.tile([128, 512], f32, name=f"psq{j}", tag="psq")
                psq01[j] = psq
                gi, r = _qgroup_of(j)
                for k in range(KC):
                    nc.tensor.matmul(
                        psq[:], wt[:, k, :], qtg[gi][:, k, r:r + 512],
                        start=(k == 0), stop=(k == KC - 1),
                    )

            def qcopy_early(j, sqq):
                sl = slice(j * 512, (j + 1) * 512)
                nc.scalar.copy(QT[:, sl], psq01[j][:])
                nc.scalar.activation(sqq[:], psq01[j][:], AF.Square)

            qproj_early(0)
            qcopy_early(0, sqqA)

            # broadcast 1/|D| across partitions and scale D straight out of
            # the projection PSUM (each psd chunk dies at its multiply)
            for ci, (off, ln) in enumerate(d_chunks):
                sl = slice(off, off + ln)
                psb = psB.tile([128, 512], f32, name="psb", tag="psb")
                nc.tensor.matmul(
                    psb[:, :ln], oneeighth[:], invnD8[:, sl], start=True, stop=True
                )
                bc = bc_pool.tile([128, 512], f32, name="bcast_sb", tag="bc")
                if ci < 3:
                    nc.scalar.copy(bc[:, :ln], psb[:, :ln])
                else:
                    nc.vector.tensor_scalar_mul(bc[:, :ln], psb[:, :ln], 1.0)
                nc.vector.tensor_tensor(
                    DTn[:, sl], psds[ci][:, :ln], bc[:, :ln], op=ALU.mult
                )
            qproj_early(1)
            qcopy_early(1, sqqB)

        # ---------- phase Q+S: project query chunks, sim tiles interleaved --
        # Q-projection chunk j feeds sim tiles t=4j..4j+3; chunks are traced
        # two groups ahead of their sim tiles so the PE never starves the DVE
        # reduce pipeline.  pssim is a single 5-bank tensor whose two halves
        # ping-pong between PE writes and DVE reduces.
        with (
            tc.tile_pool(name="psQS", bufs=2, space="PSUM") as psQS,
            tc.tile_pool(name="ssQ", bufs=1, space="PSUM") as ssQ,
            tc.tile_pool(name="sqQ_pool", bufs=2) as sqQ_pool,
            tc.tile_pool(name="psO", bufs=1, space="PSUM") as psO,
            tc.tile_pool(name="m_pool", bufs=8) as m_pool,
            tc.tile_pool(name="fold_pool", bufs=5) as fold_pool,
        ):
            ssq = ssQ.tile([128, NTT], f32, name="ssq")
            psout = psO.tile([BPT, NTT * DPC], f32, name="psout")

            psq_live = {}

            def project_mm(j, ks):
                gi, r = _qgroup_of(j)
                if j not in psq_live:
                    psq_live[j] = psQS.tile([128, NVH], f32, name="psq", tag="big")
                psq = psq_live[j]
                for k in ks:
                    nc.tensor.matmul(
                        psq[:, 0:512],
                        wt[:, k, :],
                        qtg[gi][:, k, r:r + 512],
                        start=(k == 0),
                        stop=(k == KC - 1),
                    )

            def qnorm(j, sq):
                # ssq matmuls + per-chunk 1/|Q| and the weighted lhsT
                for s in range(4):
                    col = j * 4 + s
                    nc.tensor.matmul(
                        ssq[:, col:col + 1],
                        sq[:, s * 128:(s + 1) * 128],
                        onescol[:],
                        start=True,
                        stop=True,
                    )
                csl = slice(j * 4, (j + 1) * 4)
                nc.scalar.activation(normQ[:, csl], ssq[:, csl], AF.Sqrt)
                nc.vector.reciprocal(invnQ[:, csl], normQ[:, csl])
                nc.vector.tensor_tensor(
                    lhsQ[:, csl, :],
                    qso[:].unsqueeze(1).broadcast_to((128, 4, BPT)),
                    invnQ[:, csl].unsqueeze(2).broadcast_to((128, 4, BPT)),
                    op=ALU.mult,
                )

            def project(j):
                sl = slice(j * 512, (j + 1) * 512)
                psq = psq_live.pop(j)
                nc.scalar.copy(QT[:, sl], psq[:, 0:512])
                sq = sqQ_pool.tile([128, 512], sq_dt, name="sqq", tag="sqq")
                nc.scalar.activation(sq[:], psq[:, 0:512], AF.Square)
                qnorm(j, sq)

            def psout_mm(t, mall):
                nc.tensor.matmul(
                    psout[:, t * DPC:(t + 1) * DPC],
                    lhsQ[:, t, :],
                    mall[:],
                    start=True,
                    stop=True,
                )

            def fold_levels(sv_tiles, g, out, tag):
                # two 2x-rate TT-max fold levels then a short 1x reduce over
                # [128, g, NV/4]; sv_tiles is a (g*NV)-wide bf16 SBUF region
                v2, v4 = NV // 2, NV // 4
                l1 = fold_pool.tile([128, g * v2], sim_dt, name=f"l1{tag}",
                                    tag=f"l1{tag}")
                sv = sv_tiles.rearrange("p (g v) -> p g v", v=NV)
                nc.vector.tensor_tensor(
                    l1[:].rearrange("p (g v) -> p g v", v=v2),
                    sv[:, :, 0:v2], sv[:, :, v2:NV], op=ALU.max,
                )
                l2 = fold_pool.tile([128, g * v4], sim_dt, name=f"l2{tag}",
                                    tag=f"l2{tag}")
                lv = l1[:].rearrange("p (g v) -> p g v", v=v2)
                nc.vector.tensor_tensor(
                    l2[:].rearrange("p (g v) -> p g v", v=v4),
                    lv[:, :, 0:v4], lv[:, :, v4:v2], op=ALU.max,
                )
                nc.vector.reduce_max(
                    out, l2[:].rearrange("p (g v) -> p g v", v=v4), axis=AX.X
                )

            def fold_finish(t, kind, sbt, mall):
                # issued a couple of tiles late so the DVE's inputs are
                # always ready (no cross-engine just-in-time stalls)
                if kind == "F":
                    fold_levels(sbt[:], DPC, mall[:], "f")
                else:  # H: h1 only (docs 8..15)
                    fold_levels(sbt[:], DPC // 2, mall[:, DPC // 2:DPC], "h")
                psout_mm(t, mall)

            def simtile(t, mid=None):
                lq = QT[:, t * 128:(t + 1) * 128]
                kind = _tile_kind(t)
                mall = m_pool.tile([128, DPC], f32, name="mall", tag="mall")
                if kind == "F":
                    sbt = fold_pool.tile([128, NVT], sim_dt, name="sbt", tag="sbt")
                elif kind == "H":
                    sbt = fold_pool.tile([128, NVH], sim_dt, name="sbh", tag="sbh")
                for h in range(2):
                    base = h * NVH
                    ps = psQS.tile([128, NVH], f32, name="pssim", tag="big")
                    for (off, ln) in s_chunks:
                        nc.tensor.matmul(
                            ps[:, off:off + ln],
                            lq,
                            DTn[:, base + off:base + off + ln],
                            start=True,
                            stop=True,
                        )
                    if kind == "F" or (kind == "H" and h == 1):
                        # evict to SBUF bf16 on ACT; DVE folds later
                        off0 = base if kind == "F" else 0
                        nc.scalar.copy(sbt[:, off0:off0 + NVH], ps[:])
                    else:
                        nc.vector.reduce_max(
                            mall[:, h * (DPC // 2):(h + 1) * (DPC // 2)],
                            ps[:].rearrange("p (g v) -> p g v", v=NV),
                            axis=AX.X,
                        )
                    if mid:
                        mid(h)
                if kind == "D":
                    psout_mm(t, mall)
                    return None
                return (t, kind, sbt, mall)

            # chunks 0/1 were projected + copied during phase D; finish norms
            qnorm(0, sqqA)
            pending = []

            def run_tile(t, mid=None):
                p = simtile(t, mid)
                if p is not None:
                    pending.append(p)
                depth = 3 if t < 29 else 1
                while len(pending) > depth:
                    fold_finish(*pending.pop(0))

            run_tile(0)
            qnorm(1, sqqB)
            run_tile(1)
            for t in range(2, NTT):
                j, ti = divmod(t, 4)
                mid = None
                if ti == 2 and j + 2 < NQCH:
                    # half the next-next chunk's projection between each sim
                    # half so the PE pause never starves the DVE
                    def mid(h, _j=j + 2):
                        project_mm(_j, range(3 * h, 3 * h + 3))

                run_tile(t, mid)
                if ti == 2 and j + 2 < NQCH:
                    project(j + 2)
            for p in pending:
                fold_finish(*p)
            nc.vector.tensor_scalar_mul(outstage[:], psout[:], 1.0)
            nc.sync.dma_start(
                out_d[:, :].rearrange("(t f) c -> f t c", f=BPT),
                outstage[:].rearrange("f (t c) -> f t c", c=DPC),
            )

    nc.compile()
    return nc


def _host_prep(q_hidden, d_hidden, W, d_mask):
    import ml_dtypes

    bf = ml_dtypes.bfloat16
    q = np.ascontiguousarray(np.asarray(q_hidden, dtype=np.float32))
    d = np.ascontiguousarray(np.asarray(d_hidden, dtype=np.float32))
    w = np.ascontiguousarray(np.asarray(W, dtype=np.float32))
    mask = np.asarray(d_mask, dtype=bool)

    nv = mask.sum(axis=1)
    NV = int(-(-max(int(nv.max()), 16) // 8) * 8)
    NV = min(NV, ((LD + 7) // 8) * 8)

    # per-doc gather indices: valid tokens first, padded with the first
    # valid token (duplicates never change a max)
    idx = np.zeros((B, NV), dtype=np.intp)
    for c in range(B):
        v = np.flatnonzero(mask[c])
        row = np.full(NV, v[0], dtype=np.intp)
        row[:min(len(v), NV)] = v[:NV]
        idx[c] = row

    dG = d[np.arange(B)[:, None], idx, :]          # [B, NV, HID]

    qT = np.ascontiguousarray(q.reshape(TQ, HID).T.astype(bf))   # [HID, TQ]
    # W.T rearranged so the [128, KC, DIM] SBUF tile is one contiguous DMA:
    # wTp[p, k, d] = W[d, k*128+p]
    wT = np.ascontiguousarray(
        w.T.reshape(KC, 128, DIM).transpose(1, 0, 2).astype(bf)
    )
    f8 = ml_dtypes.float8_e4m3
    dT_cores = []
    for m in range(NCORES):
        blk = dG[m * DPC:(m + 1) * DPC].reshape(DPC * NV, HID)
        dT_cores.append(np.ascontiguousarray(blk.T.astype(f8)))  # [HID, DPC*NV]

    qso = np.zeros((128, 128 // LQ), dtype=np.float32)
    for p in range(128):
        qso[p, p // LQ] = 1.0
    onescol = np.ones((128, 1), dtype=bf)
    return NV, qT, wT, dT_cores, qso, onescol


def kernel(q_hidden, d_hidden, W, d_mask):
    from concourse.bass_utils import run_bass_kernel_spmd

    NV, qT, wT, dT_cores, qso, onescol = _host_prep(
        q_hidden, d_hidden, W, d_mask
    )
    nc = _build_program(NV)

    in_maps = [
        {
            "qT": qT,
            "dT": dT_cores[m],
            "wT": wT,
            "qso": qso,
            "onescol": onescol,
        }
        for m in range(NCORES)
    ]
    res = run_bass_kernel_spmd(nc, in_maps, core_ids=list(range(NCORES)))
    out = np.concatenate(
        [res.results[m]["out"] for m in range(NCORES)], axis=1
    )
    return np.ascontiguousarray(out.astype(np.float32))


# revision 49
# speedup vs baseline: 1.0241x; 1.0188x over previous
"""ColBERT intra-batch MaxSim scoring kernel for 8 Trainium2 NeuronCores.

Math (see reference):
  Q = l2norm(q_hidden @ W.T)                       [B, LQ, DIM]
  D = l2norm(d_hidden @ W.T); D masked             [B, LD, DIM]
  sim[b,c,q,k] = Q[b,q]·D[c,k]; masked k -> -inf
  out[b,c] = sum_q max_k sim

Sharding: docs (dim c) are sharded 16-per-core; q_hidden/W replicated.
Each core computes its [B, 16] slice of the score matrix.

Device-side structure:
  * Host pre-transposes activations to [HID, tokens] and converts to bf16
    (halves HBM traffic; verified 9.6e-4 rel err vs 2e-2 budget).
  * The doc mask is folded away on the host: each doc's valid tokens are
    gathered to the front and the tail is padded with copies of the doc's
    first valid token, so the device kernel needs no masking.
  * All input DMAs are issued on the sync queue in priority order (wt,
    dT chunks, then qT column groups).  One HWDGE ring = strict FIFO, so
    dT gets full bandwidth first and the doc pipeline starts ~15us in;
    qT groups trickle in behind at the rate the sim tiles consume them.
  * Q is NOT normalized before the sim matmul: max_k is invariant under a
    positive per-query scale, so 1/|Q| is folded into the block-ones
    lhsT of the final query-sum matmul.
  * D norms, chunk-pipelined: Square (ACT) -> M=8 ones-matmul sumsq (PE,
    full rate) -> approx reciprocal (DVE, straight from PSUM) -> sqrt with
    free bf16 cast (ACT) -> K=8 ones broadcast matmul (PE) -> multiply
    (DVE).  dT ships as fp8-e4m3 (d_hidden is N(0,1) — 6.7e-3 rel err
    total, vs the 2e-2 budget), halving the head-critical DMA.
  * Sim phase: with two PSUM slots a tile is bounded by its two halves'
    consumers when both land on one engine, so most tiles are "hybrid":
    half h0 -> DVE direct grouped reduce_max; half h1 -> ACT copy to SBUF
    bf16, folded 2 tiles later on the DVE via two 2x-rate TT-max levels +
    a short reduce (delayed so the DVE never waits on ACT).  Interleaved
    full-fold tiles shift load toward ACT to balance the engines.
  * ACT spline tables for Square/Sqrt are warmed at t=0 so the first
    normalize step doesn't eat the ~2.6us table-load latency.
"""

import os

import numpy as np

B, LQ, LD, HID, DIM = 128, 32, 256, 768, 128
NCORES = 8
DPC = B // NCORES          # docs per core
TQ = B * LQ                # total query tokens
KC = HID // 128            # contraction chunks for the projection

SIM_MODE = os.environ.get("KERNEL_SIM_MODE", "bf16")
# With only two PSUM slots, a tile's wall time is bounded by its two
# halves' consumers when they land on ONE engine (2 DVE reduces for a
# direct tile, 2 ACT converts for a fold tile).  Hybrid tiles send h0 to
# the DVE (direct reduce) and h1 to ACT (convert + delayed DVE TT-max
# fold), so the two consumers overlap.  A few full-fold tiles rebalance
# total load toward ACT.  D = direct, H = hybrid, F = full-fold.
FOLD = os.environ.get("KERNEL_FOLD", "1") == "1"


def _tile_kind(t):
    if not FOLD:
        return "D"
    if t < 2:
        return "D"
    # all-hybrid: ACT runs far under saturation (~44us vs DVE ~78us), so
    # converts are always ready and the DVE never stalls on cross-engine
    # coupling; the window is purely DVE-paced at ~2.4us/tile.  Mixing in
    # full-fold tiles was tried at 15/15 and 7/23 — both regressed (their
    # two serial ACT converts couple the PSUM slot pipeline).
    return "H"

# qT column groups, in DMA priority order (first groups smaller so the
# first sim tiles can start as early as possible)
QGROUPS = [(0, 512), (512, 512), (1024, 1024), (2048, 1024), (3072, 1024)]


def _chunks(total, step):
    """[(off, len)] cut at `step` boundaries — a matmul's PSUM output must
    stay inside a single 512-float bank, so chunks may never straddle one."""
    return [(o, min(step, total - o)) for o in range(0, total, step)]


def _qgroup_of(j):
    """(group index, column offset within group) for 512-col chunk j."""
    off = j * 512
    for gi, (go, gw) in enumerate(QGROUPS):
        if go <= off < go + gw:
            return gi, off - go
    raise ValueError(j)


def _build_program(NV):
    import concourse.bass as bass  # noqa: F401
    import concourse.tile as tile
    from concourse import bacc, mybir

    f32 = mybir.dt.float32
    bf16 = mybir.dt.bfloat16
    AF = mybir.ActivationFunctionType
    AX = mybir.AxisListType
    ALU = mybir.AluOpType

    proj_dt = bf16
    sim_dt = {"bf16": bf16, "f32": f32}[SIM_MODE]
    sq_dt = bf16

    NVT = DPC * NV          # compacted doc tokens per core
    NVH = NVT // 2          # half (8 docs) — one PSUM sim tile
    NQCH = TQ // 512        # q-projection column chunks
    NTT = TQ // 128         # sim lhsT tiles (query-token tiles)
    BPT = 128 // LQ         # batch entries per query-token tile
    d_chunks = _chunks(NVT, 512)   # d-projection column chunks
    s_chunks = _chunks(NVH, 512)   # sim matmul N chunks per half

    nc = bacc.Bacc(
        "TRN2",
        target_bir_lowering=False,
        debug=False,
        num_devices=NCORES,
    )

    d8 = mybir.dt.float8e4
    qT_d = nc.dram_tensor("qT", [HID, TQ], proj_dt, kind="ExternalInput")
    dT_d = nc.dram_tensor("dT", [HID, NVT], d8, kind="ExternalInput")
    wT_d = nc.dram_tensor("wT", [128, KC, DIM], proj_dt, kind="ExternalInput")
    qso_d = nc.dram_tensor("qso", [128, BPT], f32, kind="ExternalInput")
    onescol_d = nc.dram_tensor("onescol", [128, 1], sq_dt, kind="ExternalInput")
    out_d = nc.dram_tensor("out", [B, DPC], f32, kind="ExternalOutput")

    # [HID, t] rows seen as (k, p): row = k*128 + p
    qT_v = qT_d[:, :].rearrange("(k p) t -> p k t", p=128)

    with tile.TileContext(nc) as tc, tc.tile_pool(name="persist", bufs=1) as per:
        # --- constants + persistent SBUF tensors ---------------------------
        wt = per.tile([128, KC, DIM], proj_dt, name="wt")
        qso = per.tile([128, BPT], f32, name="qso")
        onescol = per.tile([128, 1], sq_dt, name="onescol")
        onescol8 = per.tile([128, 8], sq_dt, name="onescol8")
        oneeighth = per.tile([8, 128], sq_dt, name="oneeighth")
        warm = per.tile([1, 16], f32, name="warm")
        QT = per.tile([128, TQ], sim_dt, name="QT")       # q-proj [d, t] unnormalized
        DTn = per.tile([128, NVT], sim_dt, name="DTn")    # normalized d-proj
        invnQ = per.tile([128, NTT], f32, name="invnQ")   # 1/|Q| per query token
        normQ = per.tile([128, NTT], f32, name="normQ")
        lhsQ = per.tile([128, NTT, BPT], f32, name="lhsQ")  # blockones * 1/|Q|
        rowtmp8 = per.tile([8, NVT], f32, name="rowtmp8")    # 1/ssq, 8 rows
        invnD8 = per.tile([8, NVT], sq_dt, name="invnD8")    # 1/|D| bf16, 8 rows
        outstage = per.tile([BPT, NTT * DPC], f32, name="outstage")
        sqqA = per.tile([128, 512], sq_dt, name="sqqA")
        sqqB = per.tile([128, 512], sq_dt, name="sqqB")
        dts = [per.tile([128, NVT], d8, name=f"dt{k}") for k in range(KC)]
        qtg = [
            per.tile([128, KC, gw], proj_dt, name=f"qtg{gi}")
            for gi, (_, gw) in enumerate(QGROUPS)
        ]

        # input DMAs: one ring (sync), strict priority order
        nc.sync.dma_start(wt[:], wT_d[:, :, :])
        # chunk-0 columns of every k first (tiny), so the norm chain's
        # first link starts ~7us earlier; bulk columns right behind
        for k in range(KC):
            nc.sync.dma_start(dts[k][:, 0:512], dT_d[k * 128:(k + 1) * 128, 0:512])
        for k in range(KC):
            nc.sync.dma_start(dts[k][:, 512:NVT], dT_d[k * 128:(k + 1) * 128, 512:NVT])
        for gi, (go, gw) in enumerate(QGROUPS):
            nc.sync.dma_start(qtg[gi][:, :, :], qT_v[:, :, go:go + gw])
        # tiny constants off the critical ring (SWDGE)
        nc.gpsimd.dma_start(qso[:], qso_d[:, :])
        nc.gpsimd.dma_start(onescol[:], onescol_d[:, :])

        # warm the ACT spline tables while DMAs are in flight
        nc.vector.memset(warm[:], 1.0)
        nc.vector.memset(onescol8[:], 1.0)
        nc.vector.memset(oneeighth[:], 0.125)
        nc.scalar.activation(warm[:], warm[:], AF.Square)
        nc.scalar.activation(warm[:], warm[:], AF.Sqrt)

        # ---------------- phase D: project doc tokens ----------------------
        # k-outer accumulation into one wide PSUM tensor so compute starts
        # as soon as the first dT k-chunk lands.
        with (
            tc.tile_pool(name="psD", bufs=1, space="PSUM") as psD,
            tc.tile_pool(name="ssD", bufs=1, space="PSUM") as ssD,
            tc.tile_pool(name="sqD_pool", bufs=2) as sqD_pool,
            tc.tile_pool(name="bc_pool", bufs=2) as bc_pool,
            tc.tile_pool(name="psB", bufs=1, space="PSUM") as psB,
            tc.tile_pool(name="psqP", bufs=1, space="PSUM") as psqP,
        ):
            # psd split per 512-chunk so each chunk's PSUM bank frees right
            # after its DTn multiply (the Q-projection PSUM reuses them)
            psds = [
                psD.tile([128, ln], f32, name=f"psd{ci}")
                for ci, (off, ln) in enumerate(d_chunks)
            ]
            # first group = one chunk (it only needs the c0-priority DMA
            # pieces), and each group's norm-chain steps are emitted right
            # after its k-sweeps so the PE FIFO reaches ssd(c0) immediately
            cgroups = [[0], [1, 2], [3, 4]]

            def chain_step(ci):
                off, ln = d_chunks[ci]
                sl = slice(off, off + ln)
                sq = sqD_pool.tile([128, 512], sq_dt, name="sqd", tag="sq")
                nc.scalar.activation(sq[:, :ln], psds[ci][:, :ln], AF.Square)
                ssd = ssD.tile([8, 512], f32, name="ssd", tag="ssd")
                nc.tensor.matmul(
                    ssd[:, :ln], onescol8[:], sq[:, :ln], start=True, stop=True
                )
                nc.vector.reciprocal_approx_fast(
                    rowtmp8[:, sl], ssd[:, :ln]
                )
                nc.scalar.activation(invnD8[:, sl], rowtmp8[:, sl], AF.Sqrt)

            for cg in cgroups:
                for k in range(KC):
                    for ci in cg:
                        off, ln = d_chunks[ci]
                        nc.tensor.matmul(
                            psds[ci][:, :ln],
                            wt[:, k, :],
                            dts[k][:, off:off + ln],
                            start=(k == 0),
                            stop=(k == KC - 1),
                        )
                for ci in cg:
                    chain_step(ci)

            # Q-projection chunk 0 into its own PSUM bank (runs in the PE's
            # DMA-wait gaps); its QT copy + square land on ACT just before
            # the bc copies so sim tile 0 can fire the moment DTn is done
            psq01 = {}

            def qproj_early(j):
                psq = psqP.tile([128, 512], f32, name=f"psq{j}", tag="psq")
                psq01[j] = psq
                gi, r = _qgroup_of(j)
                for k in range(KC):
                    nc.tensor.matmul(
                        psq[:], wt[:, k, :], qtg[gi][:, k, r:r + 512],
                        start=(k == 0), stop=(k == KC - 1),
                    )

            def qcopy_early(j, sqq):
                sl = slice(j * 512, (j + 1) * 512)
                nc.scalar.copy(QT[:, sl], psq01[j][:])
                nc.scalar.activation(sqq[:], psq01[j][:], AF.Square)

            qproj_early(0)
            qcopy_early(0, sqqA)

            # broadcast 1/|D| across partitions and scale D straight out of
            # the projection PSUM (each psd chunk dies at its multiply)
            for ci, (off, ln) in enumerate(d_chunks):
                sl = slice(off, off + ln)
                psb = psB.tile([128, 512], f32, name="psb", tag="psb")
                nc.tensor.matmul(
                    psb[:, :ln], oneeighth[:], invnD8[:, sl], start=True, stop=True
                )
                bc = bc_pool.tile([128, 512], f32, name="bcast_sb", tag="bc")
                if ci < 3:
                    nc.scalar.copy(bc[:, :ln], psb[:, :ln])
                else:
                    nc.vector.tensor_scalar_mul(bc[:, :ln], psb[:, :ln], 1.0)
                nc.vector.tensor_tensor(
                    DTn[:, sl], psds[ci][:, :ln], bc[:, :ln], op=ALU.mult
                )
            qproj_early(1)
            qcopy_early(1, sqqB)

        # ---------- phase Q+S: project query chunks, sim tiles interleaved --
        # Q-projection chunk j feeds sim tiles t=4j..4j+3; chunks are traced
        # two groups ahead of their sim tiles so the PE never starves the DVE
        # reduce pipeline.  pssim is a single 5-bank tensor whose two halves
        # ping-pong between PE writes and DVE reduces.
        with (
            tc.tile_pool(name="psQS", bufs=2, space="PSUM") as psQS,
            tc.tile_pool(name="ssQ", bufs=1, space="PSUM") as ssQ,
            tc.tile_pool(name="sqQ_pool", bufs=2) as sqQ_pool,
            tc.tile_pool(name="psO", bufs=1, space="PSUM") as psO,
            tc.tile_pool(name="m_pool", bufs=8) as m_pool,
            tc.tile_pool(name="fold_pool", bufs=5) as fold_pool,
        ):
            ssq = ssQ.tile([128, NTT], f32, name="ssq")
            psout = psO.tile([BPT, NTT * DPC], f32, name="psout")

            psq_live = {}

            def project_mm(j, ks):
                gi, r = _qgroup_of(j)
                if j not in psq_live:
                    psq_live[j] = psQS.tile([128, NVH], f32, name="psq", tag="big")
                psq = psq_live[j]
                for k in ks:
                    nc.tensor.matmul(
                        psq[:, 0:512],
                        wt[:, k, :],
                        qtg[gi][:, k, r:r + 512],
                        start=(k == 0),
                        stop=(k == KC - 1),
                    )

            def qnorm(j, sq):
                # ssq matmuls + per-chunk 1/|Q| and the weighted lhsT
                for s in range(4):
                    col = j * 4 + s
                    nc.tensor.matmul(
                        ssq[:, col:col + 1],
                        sq[:, s * 128:(s + 1) * 128],
                        onescol[:],
                        start=True,
                        stop=True,
                    )
                csl = slice(j * 4, (j + 1) * 4)
                nc.scalar.activation(normQ[:, csl], ssq[:, csl], AF.Sqrt)
                nc.vector.reciprocal(invnQ[:, csl], normQ[:, csl])
                nc.vector.tensor_tensor(
                    lhsQ[:, csl, :],
                    qso[:].unsqueeze(1).broadcast_to((128, 4, BPT)),
                    invnQ[:, csl].unsqueeze(2).broadcast_to((128, 4, BPT)),
                    op=ALU.mult,
                )

            def project(j):
                sl = slice(j * 512, (j + 1) * 512)
                psq = psq_live.pop(j)
                nc.scalar.copy(QT[:, sl], psq[:, 0:512])
                sq = sqQ_pool.tile([128, 512], sq_dt, name="sqq", tag="sqq")
                nc.scalar.activation(sq[:], psq[:, 0:512], AF.Square)
                qnorm(j, sq)

            def psout_mm(t, mall):
                nc.tensor.matmul(
                    psout[:, t * DPC:(t + 1) * DPC],
                    lhsQ[:, t, :],
                    mall[:],
                    start=True,
                    stop=True,
                )

            def fold_levels(sv_tiles, g, out, tag):
                # two 2x-rate TT-max fold levels then a short 1x reduce over
                # [128, g, NV/4]; sv_tiles is a (g*NV)-wide bf16 SBUF region
                v2, v4 = NV // 2, NV // 4
                l1 = fold_pool.tile([128, g * v2], sim_dt, name=f"l1{tag}",
                                    tag=f"l1{tag}")
                sv = sv_tiles.rearrange("p (g v) -> p g v", v=NV)
                nc.vector.tensor_tensor(
                    l1[:].rearrange("p (g v) -> p g v", v=v2),
                    sv[:, :, 0:v2], sv[:, :, v2:NV], op=ALU.max,
                )
                l2 = fold_pool.tile([128, g * v4], sim_dt, name=f"l2{tag}",
                                    tag=f"l2{tag}")
                lv = l1[:].rearrange("p (g v) -> p g v", v=v2)
                nc.vector.tensor_tensor(
                    l2[:].rearrange("p (g v) -> p g v", v=v4),
                    lv[:, :, 0:v4], lv[:, :, v4:v2], op=ALU.max,
                )
                nc.vector.reduce_max(
                    out, l2[:].rearrange("p (g v) -> p g v", v=v4), axis=AX.X
                )

            def fold_finish(t, kind, sbt, mall):
                # issued a couple of tiles late so the DVE's inputs are
                # always ready (no cross-engine just-in-time stalls)
                if kind == "F":
                    fold_levels(sbt[:], DPC, mall[:], "f")
                else:  # H: h1 only (docs 8..15)
                    fold_levels(sbt[:], DPC // 2, mall[:, DPC // 2:DPC], "h")
                psout_mm(t, mall)

            def simtile(t, mid=None):
                lq = QT[:, t * 128:(t + 1) * 128]
                kind = _tile_kind(t)
                mall = m_pool.tile([128, DPC], f32, name="mall", tag="mall")
                if kind == "F":
                    sbt = fold_pool.tile([128, NVT], sim_dt, name="sbt", tag="sbt")
                elif kind == "H":
                    sbt = fold_pool.tile([128, NVH], sim_dt, name="sbh", tag="sbh")
                for h in range(2):
                    base = h * NVH
                    ps = psQS.tile([128, NVH], f32, name="pssim", tag="big")
                    for (off, ln) in s_chunks:
                        nc.tensor.matmul(
                            ps[:, off:off + ln],
                            lq,
                            DTn[:, base + off:base + off + ln],
                            start=True,
                            stop=True,
                        )
                    if kind == "F" or (kind == "H" and h == 1):
                        # evict to SBUF bf16 on ACT; DVE folds later
                        off0 = base if kind == "F" else 0
                        nc.scalar.copy(sbt[:, off0:off0 + NVH], ps[:])
                    else:
                        nc.vector.reduce_max(
                            mall[:, h * (DPC // 2):(h + 1) * (DPC // 2)],
                            ps[:].rearrange("p (g v) -> p g v", v=NV),
                            axis=AX.X,
                        )
                    if mid:
                        mid(h)
                if kind == "D":
                    psout_mm(t, mall)
                    return None
                return (t, kind, sbt, mall)

            # chunks 0/1 were projected + copied during phase D; finish norms
            qnorm(0, sqqA)
            pending = []

            def run_tile(t, mid=None):
                p = simtile(t, mid)
                if p is not None:
                    pending.append(p)
                depth = 3 if t < 29 else 1
                while len(pending) > depth:
                    fold_finish(*pending.pop(0))

            run_tile(0)
            qnorm(1, sqqB)
            run_tile(1)
            for t in range(2, NTT):
                j, ti = divmod(t, 4)
                mid = None
                if ti == 2 and j + 2 < NQCH:
                    # half the next-next chunk's projection between each sim
                    # half so the PE pause never starves the DVE
                    def mid(h, _j=j + 2):
                        project_mm(_j, range(3 * h, 3 * h + 3))

                run_tile(t, mid)
                if ti == 2 and j + 2 < NQCH:
                    project(j + 2)
            for p in pending:
                fold_finish(*p)
            nc.vector.tensor_scalar_mul(outstage[:], psout[:], 1.0)
            nc.sync.dma_start(
                out_d[:, :].rearrange("(t f) c -> f t c", f=BPT),
                outstage[:].rearrange("f (t c) -> f t c", c=DPC),
            )

    nc.compile()
    return nc


def _host_prep(q_hidden, d_hidden, W, d_mask):
    import ml_dtypes

    bf = ml_dtypes.bfloat16
    q = np.ascontiguousarray(np.asarray(q_hidden, dtype=np.float32))
    d = np.ascontiguousarray(np.asarray(d_hidden, dtype=np.float32))
    w = np.ascontiguousarray(np.asarray(W, dtype=np.float32))
    mask = np.asarray(d_mask, dtype=bool)

    nv = mask.sum(axis=1)
    NV = int(-(-max(int(nv.max()), 16) // 8) * 8)
    NV = min(NV, ((LD + 7) // 8) * 8)

    # per-doc gather indices: valid tokens first, padded with the first
    # valid token (duplicates never change a max)
    idx = np.zeros((B, NV), dtype=np.intp)
    for c in range(B):
        v = np.flatnonzero(mask[c])
        row = np.full(NV, v[0], dtype=np.intp)
        row[:min(len(v), NV)] = v[:NV]
        idx[c] = row

    dG = d[np.arange(B)[:, None], idx, :]          # [B, NV, HID]

    qT = np.ascontiguousarray(q.reshape(TQ, HID).T.astype(bf))   # [HID, TQ]
    # W.T rearranged so the [128, KC, DIM] SBUF tile is one contiguous DMA:
    # wTp[p, k, d] = W[d, k*128+p]
    wT = np.ascontiguousarray(
        w.T.reshape(KC, 128, DIM).transpose(1, 0, 2).astype(bf)
    )
    f8 = ml_dtypes.float8_e4m3
    dT_cores = []
    for m in range(NCORES):
        blk = dG[m * DPC:(m + 1) * DPC].reshape(DPC * NV, HID)
        dT_cores.append(np.ascontiguousarray(blk.T.astype(f8)))  # [HID, DPC*NV]

    qso = np.zeros((128, 128 // LQ), dtype=np.float32)
    for p in range(128):
        qso[p, p // LQ] = 1.0
    onescol = np.ones((128, 1), dtype=bf)
    return NV, qT, wT, dT_cores, qso, onescol


def kernel(q_hidden, d_hidden, W, d_mask):
    from concourse.bass_utils import run_bass_kernel_spmd

    NV, qT, wT, dT_cores, qso, onescol = _host_prep(
        q_hidden, d_hidden, W, d_mask
    )
    nc = _build_program(NV)

    in_maps = [
        {
            "qT": qT,
            "dT": dT_cores[m],
            "wT": wT,
            "qso": qso,
            "onescol": onescol,
        }
        for m in range(NCORES)
    ]
    res = run_bass_kernel_spmd(nc, in_maps, core_ids=list(range(NCORES)))
    out = np.concatenate(
        [res.results[m]["out"] for m in range(NCORES)], axis=1
    )
    return np.ascontiguousarray(out.astype(np.float32))
